# revision 1
# baseline (speedup 1.0000x reference)
"""CopyGenerator kernel for Trainium2 (Bass/Tile), vocab-parallel across 8 cores.

res[t,b,v] = a[b]*p_copy[b,t,v] + (1-a[b])*p_gen[t,b,v]
  p_gen = htgt @ Wg + bg
  attn  = softmax((htgt@Wq+bq)/sqrt(D) @ (hsrc@Wq+bq).T)
  p_copy[b,t,src[s,b]] += attn[b,t,s]      (scatter == attn @ onehot(src))
  a[b]  = sigmoid(sum_t((attn@ (hsrc@Wq+bq)) @ Wf + bf) @ Wc + bc)
        = sigmoid(colsum(attn) . (hsrc[:,b,:] @ Wq@Wf@Wc) + c0)   [exact algebra]

Each core c owns vocab slice [c*4000, (c+1)*4000). The scatter is realized as a
one-hot matmul fused into the generator GEMM: per batch, one PSUM accumulation
over 5 K-chunks of 128: 4 chunks of (1-a)*htgt.T against Wg, plus 1 chunk of
a*attn.T against onehot(src - v0), built with a single int16 is_equal against an
iota row. Optional bg support via a K=1 matmul using softmax-rows-sum-to-one.
"""

import math
import numpy as np

NT, NS, B, D, V = 128, 128, 8, 512, 32000
NCORES = 8
VS = V // NCORES            # 4000 vocab columns per core
P = 128
KC = D // P                 # 4 contraction chunks of 128
NTILE = 500                 # PSUM free dim per GEMM tile (<=512 fp32)
NNT = VS // NTILE           # 8 vocab tiles per core
SQ = 1.0 / math.sqrt(D)

GEMM_DTYPE = "bf16"         # "bf16" | "f32r" | "f32"

_module_cache: dict = {}


def _build_module(bg_nonzero: bool):
    from contextlib import ExitStack

    import concourse.bass as bass
    import concourse.mybir as mybir
    import concourse.tile as tile
    from concourse import bacc
    from concourse.masks import make_identity

    f32 = mybir.dt.float32
    i16 = mybir.dt.int16
    if GEMM_DTYPE == "bf16":
        mmdt = mybir.dt.bfloat16
    elif GEMM_DTYPE == "f32r":
        mmdt = mybir.dt.float32r
    else:
        mmdt = mybir.dt.float32

    nc = bacc.Bacc(
        "TRN2",
        target_bir_lowering=False,
        debug=False,
        enable_asserts=False,
        num_devices=NCORES,
    )

    htgt_d = nc.dram_tensor("htgt", (NT, B, D), f32, kind="ExternalInput").ap()
    hsrc_d = nc.dram_tensor("hsrc", (NS, B, D), f32, kind="ExternalInput").ap()
    srcsh_d = nc.dram_tensor("srcsh", (NS, B), i16, kind="ExternalInput").ap()
    wq_d = nc.dram_tensor("wq", (D, D), f32, kind="ExternalInput").ap()
    wg_d = nc.dram_tensor("wg", (D, VS), f32, kind="ExternalInput").ap()
    w3_d = nc.dram_tensor("w3", (D,), f32, kind="ExternalInput").ap()
    bq_d = nc.dram_tensor("bq", (D,), f32, kind="ExternalInput").ap()
    bg_d = nc.dram_tensor("bg", (VS,), f32, kind="ExternalInput").ap()
    c0v_d = nc.dram_tensor("c0v", (P,), f32, kind="ExternalInput").ap()
    out_d = nc.dram_tensor("out", (NT, B, VS), f32, kind="ExternalOutput").ap()

    Id = mybir.ActivationFunctionType.Identity
    Exp = mybir.ActivationFunctionType.Exp
    Sigmoid = mybir.ActivationFunctionType.Sigmoid
    is_equal = mybir.AluOpType.is_equal
    X = mybir.AxisListType.X

    with tile.TileContext(nc) as tc, ExitStack() as ctx:
        sb = ctx.enter_context(tc.tile_pool(name="sb", bufs=1))

        # ---- persistent constants ----
        ident_f = sb.tile([P, P], f32)
        make_identity(nc, ident_f[:])
        ones_m = sb.tile([P, 1], mmdt)
        nc.vector.memset(ones_m[:], 1.0)
        ones_f = sb.tile([P, 1], f32)
        nc.vector.memset(ones_f[:], 1.0)
        iota_sb = sb.tile([P, VS], i16)
        nc.gpsimd.iota(iota_sb[:], pattern=[[1, VS]], base=0, channel_multiplier=0)
        # DVE observer: the per-batch is_equal (TensorTensor encodes only ONE
        # sync wait on trn2) must never need a Pool/DMA wait itself.
        nc.vector.tensor_copy(iota_sb[:, 0:1], iota_sb[:, 0:1])

        srcsh_sb = sb.tile([P, B], i16)
        nc.sync.dma_start(srcsh_sb[:], srcsh_d[:, :])
        nc.vector.tensor_copy(srcsh_sb[:, 0:1], srcsh_sb[:, 0:1])

        w3_f = sb.tile([P, KC], f32)
        nc.sync.dma_start(w3_f[:], w3_d.rearrange("(o p) -> p o", p=P))
        w3_m = sb.tile([P, KC], mmdt)
        nc.vector.tensor_copy(w3_m[:], w3_f[:])
        bq_t = sb.tile([P, KC], f32)
        nc.sync.dma_start(bq_t[:], bq_d.rearrange("(o p) -> p o", p=P))
        bqs_t = sb.tile([P, KC], f32)
        nc.vector.tensor_scalar_mul(bqs_t[:], bq_t[:], SQ)
        c0v_sb = sb.tile([P, 1], f32)
        nc.sync.dma_start(c0v_sb[:], c0v_d[:, None])

        wq_m = sb.tile([P, KC, D], mmdt)
        wg_m = sb.tile([P, KC, VS], mmdt)
        htgtT = sb.tile([P, KC, B, P], mmdt)   # [d_in, kc, b, t]
        hsrcT = sb.tile([P, KC, B, P], mmdt)   # [d_in, kc, b, s]
        if bg_nonzero:
            bg_m = sb.tile([1, VS], mmdt)

        with tc.tile_pool(name="ppA", bufs=2, space="PSUM") as ppA:
            # ---- staging loads (single big DMAs, no slot reuse -> no waits) ----
            with tc.tile_pool(name="stg", bufs=1) as stg:
                wq_f = stg.tile([P, KC, D], f32)
                nc.sync.dma_start(wq_f[:], wq_d.rearrange("(o p) n -> p o n", p=P))
                nc.vector.tensor_copy(wq_m[:], wq_f[:])

                wg_f = stg.tile([P, KC, VS], f32)
                for kc in range(KC):
                    nc.sync.dma_start(wg_f[:, kc, :], wg_d[kc * P : (kc + 1) * P, :])
                    nc.vector.tensor_copy(wg_m[:, kc, :], wg_f[:, kc, :])

                if bg_nonzero:
                    bg_st = stg.tile([1, VS], f32)
                    nc.sync.dma_start(bg_st[:], bg_d[None, :])
                    nc.vector.tensor_copy(bg_m[:], bg_st[:])

                # htgt/hsrc: one DMA each; transpose f32 via PE, cast on copy-out
                htgt_f = stg.tile([P, B * D], f32)
                hsrc_f = stg.tile([P, B * D], f32)
                for hh in range(2):
                    hsl = slice(hh * (B // 2) * D, (hh + 1) * (B // 2) * D)
                    bsl2 = slice(hh * (B // 2), (hh + 1) * (B // 2))
                    nc.sync.dma_start(
                        htgt_f[:, hsl],
                        htgt_d[:, bsl2, :].rearrange("t b d -> t (b d)"),
                    )
                    nc.sync.dma_start(
                        hsrc_f[:, hsl],
                        hsrc_d[:, bsl2, :].rearrange("s b d -> s (b d)"),
                    )
                for b in range(B):
                    for src_t, dstT in ((htgt_f, htgtT), (hsrc_f, hsrcT)):
                        for kc in range(KC):
                            tp = ppA.tile([P, P], f32, tag="trA")
                            nc.tensor.transpose(
                                tp[:],
                                src_t[:, b * D + kc * P : b * D + (kc + 1) * P],
                                ident_f[:],
                            )
                            nc.vector.tensor_copy(dstT[:, kc, b, :], tp[:])

            # ---- projections qT/kT for all batches ----
            qT = sb.tile([P, KC, B, P], mmdt)  # [d_out, mc, b, t] (q pre-scaled 1/sqrt(D))
            kT = sb.tile([P, KC, B, P], mmdt)
            for srcT, dstT, scale, bias in (
                (htgtT, qT, SQ, bqs_t),
                (hsrcT, kT, 1.0, bq_t),
            ):
                for mc in range(KC):
                    for h2 in range(2):
                        bsl = slice(4 * h2, 4 * h2 + 4)
                        ps = ppA.tile([P, 512], f32, tag="proj")
                        for kc in range(KC):
                            nc.tensor.matmul(
                                ps[:],
                                lhsT=wq_m[:, kc, mc * P : (mc + 1) * P],
                                rhs=srcT[:, kc, bsl, :],
                                start=(kc == 0),
                                stop=(kc == KC - 1),
                            )
                        nc.scalar.activation(
                            dstT[:, mc, bsl, :],
                            ps[:].rearrange("p (b t) -> p b t", b=4),
                            Id,
                            bias=bias[:, mc : mc + 1],
                            scale=scale,
                        )

            # ---- attention + gate pieces per batch ----
            attn_all = sb.tile([P, B, P], mmdt)  # [t, b, s] normalized softmax
            t_all = sb.tile([P, B], f32)         # colsum(attn) * (hsrc@w3 + c0/NT)
            for b in range(B):
                lg = ppA.tile([P, P], f32, tag="logits")
                for kc in range(KC):
                    nc.tensor.matmul(
                        lg[:],
                        lhsT=qT[:, kc, b, :],
                        rhs=kT[:, kc, b, :],
                        start=(kc == 0),
                        stop=(kc == KC - 1),
                    )
                negmax = sb.tile([P, 1], f32, tag="negmax", bufs=2)
                nc.vector.tensor_reduce(
                    negmax[:], lg[:], axis=X, op=mybir.AluOpType.max, negate=True
                )
                rowsum = sb.tile([P, 1], f32, tag="rowsum", bufs=2)
                attn_e = sb.tile([P, P], mmdt, tag="attn_e", bufs=2)
                nc.scalar.activation(
                    attn_e[:], lg[:], Exp, bias=negmax[:], scale=1.0,
                    accum_out=rowsum[:],
                )
                rinv = sb.tile([P, 1], f32, tag="rinv", bufs=2)
                nc.vector.reciprocal(rinv[:], rowsum[:])
                nc.vector.tensor_scalar_mul(attn_all[:, b, :], attn_e[:], rinv[:])

                asum_ps = ppA.tile([P, 1], f32, tag="small")
                nc.tensor.matmul(
                    asum_ps[:], lhsT=attn_all[:, b, :], rhs=ones_m[:],
                    start=True, stop=True,
                )
                asum_sb = sb.tile([P, 1], f32, tag="asum", bufs=2)
                nc.vector.tensor_copy(asum_sb[:], asum_ps[:])

                hv_ps = ppA.tile([P, 1], f32, tag="small")
                for kc in range(KC):
                    nc.tensor.matmul(
                        hv_ps[:],
                        lhsT=hsrcT[:, kc, b, :],
                        rhs=w3_m[:, kc : kc + 1],
                        start=(kc == 0),
                        stop=(kc == KC - 1),
                    )
                hv_sb = sb.tile([P, 1], f32, tag="hv", bufs=2)
                nc.scalar.activation(hv_sb[:], hv_ps[:], Id, bias=c0v_sb[:], scale=1.0)
                nc.vector.tensor_mul(t_all[:, b : b + 1], asum_sb[:], hv_sb[:])

            # ---- gate sigmoid + broadcast across partitions ----
            z_ps = ppA.tile([B, 1], f32, tag="small")
            nc.tensor.matmul(z_ps[:], lhsT=t_all[:], rhs=ones_f[:], start=True, stop=True)
            a_sig = sb.tile([B, 1], f32)
            nc.scalar.activation(a_sig[:], z_ps[:], Sigmoid, bias=0.0, scale=1.0)
            abc_ps = ppA.tile([P, B], f32, tag="small")
            nc.tensor.transpose(
                abc_ps[:], a_sig[:].to_broadcast([B, P]), ident_f[:B, :B]
            )
            a_bc = sb.tile([P, B], f32)
            nc.vector.tensor_copy(a_bc[:], abc_ps[:])
            om_bc = sb.tile([P, B], f32)
            nc.vector.tensor_scalar(
                om_bc[:], abc_ps[:], -1.0, 1.0,
                op0=mybir.AluOpType.mult, op1=mybir.AluOpType.add,
            )

        # ---- fused vocab GEMM per batch ----
        with tc.tile_pool(name="ppB", bufs=4, space="PSUM") as ppB, \
             tc.tile_pool(name="ppT", bufs=2, space="PSUM") as ppT, \
             tc.tile_pool(name="mn", bufs=1) as mn:
            ident_m = mn.tile([P, P], mmdt)
            nc.vector.tensor_copy(ident_m[:], ident_f[:])
            for b in range(B):
                if bg_nonzero:
                    # [1,P] row of (1-a_b) on partition 0 for the K=1 bias matmul
                    omrow = mn.tile([1, P], mmdt, tag="omrow", bufs=2)
                    nc.vector.tensor_copy(
                        omrow[:], om_bc[0:1, b : b + 1].to_broadcast([1, P])
                    )
                hT = mn.tile([P, KC + 1, P], mmdt, tag="hT", bufs=2)
                nc.vector.tensor_scalar_mul(
                    hT[:, 0:KC, :], htgtT[:, :, b, :], om_bc[:, b : b + 1]
                )
                atp = ppT.tile([P, P], mmdt, tag="atT")
                nc.tensor.transpose(atp[:], attn_all[:, b, :], ident_m[:])
                nc.vector.tensor_scalar_mul(hT[:, KC, :], atp[:], a_bc[:, b : b + 1])

                Mb = mn.tile([P, VS], mmdt, tag="Mb", bufs=2)
                nc.vector.tensor_tensor(
                    out=Mb[:],
                    in0=srcsh_sb[:, b : b + 1].to_broadcast([P, VS]),
                    in1=iota_sb[:],
                    op=is_equal,
                )

                res_h = [
                    mn.tile([P, VS // 2], f32, tag=f"res{h}", bufs=2, name=f"res_{b}_{h}")
                    for h in range(2)
                ]
                for g in range(NNT // 2):
                    pss = [
                        ppB.tile([P, NTILE], f32, tag="res_ps", name=f"res_ps_{b}_{g}_{h}")
                        for h in range(2)
                    ]
                    for j in range(KC + 1):
                        for h in range(2):
                            nt = 2 * g + h
                            vsl = slice(nt * NTILE, (nt + 1) * NTILE)
                            rhs = wg_m[:, j, vsl] if j < KC else Mb[:, vsl]
                            nc.tensor.matmul(
                                pss[h][:],
                                lhsT=hT[:, j, :],
                                rhs=rhs,
                                start=(j == 0),
                                stop=(j == KC and not bg_nonzero),
                            )
                    if bg_nonzero:
                        for h in range(2):
                            nt = 2 * g + h
                            vsl = slice(nt * NTILE, (nt + 1) * NTILE)
                            nc.tensor.matmul(
                                pss[h][:],
                                lhsT=omrow[:],
                                rhs=bg_m[:, vsl],
                                start=False,
                                stop=True,
                            )
                    for h in range(2):
                        nt = 2 * g + h
                        half = nt // (NNT // 2)
                        col = (nt % (NNT // 2)) * NTILE
                        nc.scalar.copy(res_h[half][:, col : col + NTILE], pss[h][:])
                    if g == NNT // 4 - 1:
                        nc.sync.dma_start(
                            out_d[:, b, 0 : VS // 2], res_h[0][:]
                        )
                nc.sync.dma_start(out_d[:, b, VS // 2 : VS], res_h[1][:])

    nc.compile()
    return nc


def _host_prep(inputs):
    htgt = np.ascontiguousarray(np.asarray(inputs["htgt"], dtype=np.float32))
    hsrc = np.ascontiguousarray(np.asarray(inputs["hsrc"], dtype=np.float32))
    src = np.asarray(inputs["src"]).astype(np.int64)
    Wq = np.asarray(inputs["Wq"], dtype=np.float32)
    bq = np.asarray(inputs["bq"], dtype=np.float32)
    Wf = np.asarray(inputs["Wf"], dtype=np.float32)
    bf = np.asarray(inputs["bf"], dtype=np.float32)
    Wg = np.asarray(inputs["Wg"], dtype=np.float32)
    bg = np.asarray(inputs["bg"], dtype=np.float32)
    Wc = np.asarray(inputs["Wc"], dtype=np.float32)
    bc = np.asarray(inputs["bc"], dtype=np.float32)

    # Gate weight chain (tiny): w3 = Wq@Wf@Wc, c0 = NT*(bq@Wf@Wc + bf@Wc) + bc
    wfc = (Wf.astype(np.float64) @ Wc.astype(np.float64))[:, 0]      # (D,)
    w3 = (Wq.astype(np.float64) @ wfc).astype(np.float32)            # (D,)
    c0 = float(
        NT * (bq.astype(np.float64) @ wfc)
        + NT * (bf.astype(np.float64) @ Wc.astype(np.float64)[:, 0])
        + bc[0]
    )
    c0v = np.full((P,), c0 / NT, dtype=np.float32)

    bg_nonzero = bool(np.any(bg != 0.0))

    in_maps = []
    for c in range(NCORES):
        v0 = c * VS
        in_maps.append(
            {
                "htgt": htgt,
                "hsrc": hsrc,
                "srcsh": np.ascontiguousarray((src - v0).astype(np.int16)),
                "wq": Wq,
                "wg": np.ascontiguousarray(Wg[:, v0 : v0 + VS]),
                "w3": w3,
                "bq": bq,
                "bg": np.ascontiguousarray(bg[v0 : v0 + VS]),
                "c0v": c0v,
            }
        )
    return in_maps, bg_nonzero


TRACE = False
TRACE_KW: dict = {}
LAST_RESULT = None


def kernel(**inputs) -> np.ndarray:
    global LAST_RESULT
    from concourse.bass_utils import run_bass_kernel_spmd

    in_maps, bg_nonzero = _host_prep(inputs)
    key = ("mod", bg_nonzero, GEMM_DTYPE)
    if key not in _module_cache:
        _module_cache[key] = _build_module(bg_nonzero)
    nc = _module_cache[key]

    r = run_bass_kernel_spmd(
        nc, in_maps, core_ids=list(range(NCORES)), trace=TRACE, **TRACE_KW
    )
    LAST_RESULT = r
    shards = [r.results[c]["out"] for c in range(NCORES)]
    return np.concatenate(shards, axis=2)



# revision 25
# speedup vs baseline: 1.6491x; 1.6491x over previous
"""CopyGenerator kernel for Trainium2 (Bass/Tile), vocab-parallel across 8 cores.

res[t,b,v] = a[b]*p_copy[b,t,v] + (1-a[b])*p_gen[t,b,v]
  p_gen = htgt @ Wg + bg
  attn  = softmax((htgt@Wq+bq)/sqrt(D) @ (hsrc@Wq+bq).T)
  p_copy[b,t,src[s,b]] += attn[b,t,s]      (scatter == attn @ onehot(src))
  a[b]  = sigmoid(colsum(attn) . (hsrc[:,b,:] @ Wq@Wf@Wc) + c0)   [exact algebra]

Key structure:
- logits ~ (htgt @ M' + bqq) @ hsrc.T with M' = Wq@Wq.T/sqrt(D) (softmax rows
  are invariant to per-row constants) -> no k-projection GEMM.
- Host prepares partition-major bf16 layouts -> no on-device transposes/casts.
- One-hot masks are built by GPSIMD local_scatter (dst=0; dst[p, idx[p]]=1),
  2 chunks of 2000 per batch, on the otherwise-idle Pool engine.
- The vocab GEMM for batch b is fused with attention for batches b+1/b+2
  (per-batch copy gates), so PE runs the 5-chunk PSUM-accumulated GEMM
  back-to-back from ~t=14us with zero stalls.
- Output written bf16 (rel-err budget 2e-2), upcast to f32 on the host.
"""

import math
import numpy as np

NT, NS, B, D, V = 128, 128, 8, 512, 32000
NCORES = 8
VS = V // NCORES            # 4000 vocab columns per core
P = 128
KC = D // P                 # 4 contraction chunks of 128
NTILE = 500                 # PSUM free dim per GEMM tile (<=512 fp32)
NNT = VS // NTILE           # 8 vocab tiles per core
MBC = 2000                  # local_scatter chunk width (num_elems*32 < 2^16)
SQ = 1.0 / math.sqrt(D)

_module_cache: dict = {}


def _build_module(bg_nonzero: bool):
    from contextlib import ExitStack

    import concourse.bass as bass
    import concourse.mybir as mybir
    import concourse.tile as tile
    from concourse import bacc
    from concourse.masks import make_identity

    f32 = mybir.dt.float32
    bf16 = mybir.dt.bfloat16
    i16 = mybir.dt.int16

    nc = bacc.Bacc(
        "TRN2",
        target_bir_lowering=False,
        debug=False,
        enable_asserts=False,
        num_devices=NCORES,
    )

    # Host-prepared inputs (partition-major, mostly bf16).
    htgtT_d = nc.dram_tensor("htgtT", (P, KC, B, NT), bf16, kind="ExternalInput").ap()
    hsrcT_d = nc.dram_tensor("hsrcT", (P, KC, B, NS), bf16, kind="ExternalInput").ap()
    srcidx_d = nc.dram_tensor("srcidx", (P, B, 2, 2), i16, kind="ExternalInput").ap()
    mp_d = nc.dram_tensor("mprime", (P, KC, D), bf16, kind="ExternalInput").ap()
    wg_d = nc.dram_tensor("wg", (P, KC, VS), bf16, kind="ExternalInput").ap()
    w3_d = nc.dram_tensor("w3", (P, KC), bf16, kind="ExternalInput").ap()
    bqq_d = nc.dram_tensor("bqq", (P, KC), f32, kind="ExternalInput").ap()
    bg_d = nc.dram_tensor("bg", (VS,), f32, kind="ExternalInput").ap()
    c0v_d = nc.dram_tensor("c0v", (P,), f32, kind="ExternalInput").ap()
    out_d = nc.dram_tensor("out", (NT, B, VS), bf16, kind="ExternalOutput").ap()

    Id = mybir.ActivationFunctionType.Identity
    Exp = mybir.ActivationFunctionType.Exp
    Sigmoid = mybir.ActivationFunctionType.Sigmoid
    X = mybir.AxisListType.X

    with tile.TileContext(nc) as tc, ExitStack() as ctx:
        sb = ctx.enter_context(tc.tile_pool(name="sb", bufs=1))
        pp = ctx.enter_context(tc.tile_pool(name="pp", bufs=1, space="PSUM"))
        mn = ctx.enter_context(tc.tile_pool(name="mn", bufs=1))

        # ---- input loads, most-urgent first; big tensors chunked so
        # consumers start after the first ~1.5us of DMA ----
        mp_m = sb.tile([P, KC, D], bf16)        # M' = Wq@Wq.T/sqrt(D)
        nc.sync.dma_start(mp_m[:], mp_d[:, :, :])
        htgtT = sb.tile([P, KC, B, NT], bf16)   # [d_in, kc, b, t]
        for c in range(KC):
            nc.sync.dma_start(htgtT[:, c, :, :], htgtT_d[:, c, :, :])
        srcidx = sb.tile([P, B, 2, 2], i16)
        nc.sync.dma_start(srcidx[:], srcidx_d[:, :, :, :])
        bqq_sb = sb.tile([P, KC], f32)
        nc.sync.dma_start(bqq_sb[:], bqq_d[:, :])
        w3_m = sb.tile([P, KC], bf16)
        nc.sync.dma_start(w3_m[:], w3_d[:, :])
        c0v_sb = sb.tile([P, 1], f32)
        nc.sync.dma_start(c0v_sb[:], c0v_d[:, None])
        # interleave hsrc chunks and wg quarters so both arrive just in time
        # (DMA transfers serialize on the shared DMA engines)
        hsrcT = sb.tile([P, KC, B, NS], bf16)   # [d_in, kc, b, s]
        wg_m = sb.tile([P, KC, VS], bf16)

        def wg_q(q):
            nc.sync.dma_start(
                wg_m[:, :, q * 1000 : (q + 1) * 1000],
                wg_d[:, :, q * 1000 : (q + 1) * 1000],
            )

        for c in range(2):
            nc.sync.dma_start(hsrcT[:, c, :, :], hsrcT_d[:, c, :, :])
        wg_q(0)
        for c in range(2, KC):
            nc.sync.dma_start(hsrcT[:, c, :, :], hsrcT_d[:, c, :, :])
        for q in range(1, 4):
            wg_q(q)
        if bg_nonzero:
            bg_f = sb.tile([1, VS], f32)
            nc.sync.dma_start(bg_f[:], bg_d[None, :])
            bg_m = sb.tile([1, VS], bf16)
            nc.vector.tensor_copy(bg_m[:], bg_f[:])

        # ---- PE warmup: a dependency-free accumulation chain that ramps the
        # Tensor engine to its full p-state clock (3us of continuous busy)
        # while the first DMAs land. Output is never read. Emitted first so
        # no DVE op with a DMA dependency can delay the memset.
        warm = sb.tile([P, P], bf16)
        nc.vector.memset(warm[:], 0.5)
        WARMN = 28
        psw = pp.tile([P, P], f32, tag="atT", bufs=1, name="warmps")
        for i in range(WARMN):
            nc.tensor.matmul(
                psw[:], lhsT=warm[:], rhs=warm[:],
                start=(i == 0), stop=(i == WARMN - 1),
            )

        ident_f = sb.tile([P, P], f32)
        make_identity(nc, ident_f[:])
        ident_m = sb.tile([P, P], bf16)
        nc.vector.tensor_copy(ident_m[:], ident_f[:])
        ones_m = sb.tile([P, 1], bf16)
        nc.vector.memset(ones_m[:], 1.0)
        ones_f = sb.tile([P, 1], f32)
        nc.vector.memset(ones_f[:], 1.0)

        # Pre-trigger the Activation engine's function-table loads (~1.3us
        # each) while it is idle, instead of lazily on the critical path.
        actw = sb.tile([1, 3], f32)
        nc.scalar.activation(actw[:, 0:1], ones_f[0:1, :], Id, bias=0.0, scale=1.0)
        nc.scalar.activation(actw[:, 1:2], ones_f[0:1, :], Exp, bias=0.0, scale=1.0)
        nc.scalar.activation(actw[:, 2:3], ones_f[0:1, :], Sigmoid, bias=0.0, scale=1.0)

        # DVE observer: DVE TensorScalar ops encode only one sync wait on
        # trn2, so bqq must not be a direct DMA dependency of the zT adds.
        nc.vector.tensor_copy(bqq_sb[:, 0:1], bqq_sb[:, 0:1])

        # ---- one-hot masks via GPSIMD local_scatter (Pool is idle) ----
        # mb_all[s, b, c*2000 + srcidx[s,b,c,0]] = 1, rest 0.
        ones2 = sb.tile([P, 2], bf16)
        nc.gpsimd.memset(ones2[:], 1.0)
        mb_all = sb.tile([P, B, 2, MBC], bf16)
        for b in range(B):
            for c in range(2):
                nc.gpsimd.local_scatter(
                    mb_all[:, b, c, :],
                    ones2[:],
                    srcidx[:, b, c, :],
                    channels=P,
                    num_elems=MBC,
                    num_idxs=2,
                )

        # ---- persistent SBUF state ----
        attn_s = sb.tile([P, B, NS], bf16)      # [t, b, s] softmax(logits)
        attnT_all = sb.tile([P, B, NT], bf16)   # [s, b, t] softmax, transposed
        zT = sb.tile([P, KC, B, NT], bf16)      # [d_out, co, b, t] htgt@M'+bqq
        t_all = sb.tile([P, B], f32)            # colsum(attn)*(hsrc@w3+c0/NT)
        a_bc = sb.tile([P, B], f32)             # gate a_b per partition
        om_bc = sb.tile([P, B], f32)            # 1 - a_b
        hT_all = sb.tile([P, B, KC + 1, P], bf16)
        if bg_nonzero:
            omrow = sb.tile([1, B, P], bf16)

        # ---- zT projection: z = htgt @ M' + bqq, stored transposed ----
        # h=0 half (batches 0-3) first so batch 0/1 attention can overlap
        # the h=1 half.
        def z_pass(co, h):
            bsl = slice(4 * h, 4 * h + 4)
            ps = pp.tile([P, 4 * NT], f32, tag="big", bufs=4)
            for ci in range(KC):
                nc.tensor.matmul(
                    ps[:],
                    lhsT=mp_m[:, ci, co * P : (co + 1) * P],
                    rhs=htgtT[:, ci, bsl, :].rearrange("p b t -> p (b t)"),
                    start=(ci == 0),
                    stop=(ci == KC - 1),
                )
            zsl = zT[:, co, bsl, :].rearrange("p b t -> p (b t)")
            if co % 2 == 0:
                # alternate copy engines so PSUM recycles at PE pace
                nc.vector.tensor_scalar_add(zsl, ps[:], bqq_sb[:, co : co + 1])
            else:
                nc.scalar.activation(
                    zsl, ps[:], Id, bias=bqq_sb[:, co : co + 1], scale=1.0
                )

        # ---- attention stages (emitted interleaved with the vocab GEMM) ----
        def attn_head(b):
            lg = pp.tile([P, NS], f32, tag="logits", bufs=2, name=f"lg{b}")
            for co in range(KC):
                nc.tensor.matmul(
                    lg[:],
                    lhsT=zT[:, co, b, :],
                    rhs=hsrcT[:, co, b, :],
                    start=(co == 0),
                    stop=(co == KC - 1),
                )
            negmax = sb.tile([P, 1], f32, tag="negmax", bufs=3)
            nc.vector.tensor_reduce(
                negmax[:], lg[:], axis=X, op=mybir.AluOpType.max, negate=True
            )
            rowsum = sb.tile([P, 1], f32, tag="rowsum", bufs=3)
            attn_e = sb.tile([P, NS], bf16, tag="attn_e", bufs=3)
            nc.scalar.activation(
                attn_e[:], lg[:], Exp, bias=negmax[:], scale=1.0,
                accum_out=rowsum[:],
            )
            rinv = sb.tile([P, 1], f32, tag="rinv", bufs=3)
            nc.vector.reciprocal(rinv[:], rowsum[:])
            nc.vector.tensor_scalar_mul(attn_s[:, b, :], attn_e[:], rinv[:])

        def attn_tail(b):
            # attn^T for the one-hot K-chunk (scaled by a_b in build_hT)
            atp = pp.tile([P, NT], bf16, tag="atT", bufs=1)
            nc.tensor.transpose(atp[:], attn_s[:, b, :], ident_m[:])
            nc.vector.tensor_copy(attnT_all[:, b, :], atp[:])

            # asum[s] = colsum_t attn; hv[s] = hsrc@w3 + c0/NT
            asum_ps = pp.tile([P, 1], f32, tag="small", bufs=1)
            nc.tensor.matmul(
                asum_ps[:], lhsT=attn_s[:, b, :], rhs=ones_m[:],
                start=True, stop=True,
            )
            asum_sb = sb.tile([P, 1], f32, tag="asum", bufs=2)
            nc.vector.tensor_copy(asum_sb[:], asum_ps[:])

            hv_ps = pp.tile([P, 1], f32, tag="small", bufs=1)
            for kc in range(KC):
                nc.tensor.matmul(
                    hv_ps[:],
                    lhsT=hsrcT[:, kc, b, :],
                    rhs=w3_m[:, kc : kc + 1],
                    start=(kc == 0),
                    stop=(kc == KC - 1),
                )
            hv_sb = sb.tile([P, 1], f32, tag="hv", bufs=2)
            nc.scalar.activation(hv_sb[:], hv_ps[:], Id, bias=c0v_sb[:], scale=1.0)
            nc.vector.tensor_mul(t_all[:, b : b + 1], asum_sb[:], hv_sb[:])

        def gate_zb(b):
            # a_b = sigmoid(sum_s t_all[s, b])
            zb = pp.tile([1, 1], f32, tag="small", bufs=1, name=f"zb{b}")
            nc.tensor.matmul(
                zb[:], lhsT=t_all[:, b : b + 1], rhs=ones_f[:],
                start=True, stop=True,
            )
            ab = sb.tile([1, 1], f32, tag="ab", bufs=2, name=f"ab{b}")
            nc.scalar.activation(ab[:], zb[:], Sigmoid, bias=0.0, scale=1.0)
            return ab

        def gate_abc(b, ab):
            # broadcast a_b to all 128 partitions via PE transpose
            abc = pp.tile([P, 1], f32, tag="small", bufs=1, name=f"abc{b}")
            nc.tensor.transpose(
                abc[:], ab[:].to_broadcast([1, P]), ident_f[0:1, 0:1]
            )
            nc.vector.tensor_copy(a_bc[:, b : b + 1], abc[:])
            nc.vector.tensor_scalar(
                om_bc[:, b : b + 1], abc[:], -1.0, 1.0,
                op0=mybir.AluOpType.mult, op1=mybir.AluOpType.add,
            )

        def build_hT(b):
            nc.vector.tensor_scalar_mul(
                hT_all[:, b, 0:KC, :], htgtT[:, :, b, :], om_bc[:, b : b + 1]
            )
            nc.vector.tensor_scalar_mul(
                hT_all[:, b, KC, :], attnT_all[:, b, :], a_bc[:, b : b + 1]
            )
            if bg_nonzero:
                nc.vector.tensor_copy(
                    omrow[:, b, :], om_bc[0:1, b : b + 1].to_broadcast([1, P])
                )

        # ---- prologue: zT h=0, attention heads 0/1, zT h=1 (overlaps the
        # batch-0 softmax round trip), then batch-0 tail/gate ----
        for co in range(KC):
            z_pass(co, 0)
        attn_head(0)
        attn_head(1)
        for co in range(KC):
            z_pass(co, 1)
        attn_tail(0)
        ab0 = gate_zb(0)
        gate_abc(0, ab0)
        build_hT(0)

        # ---- fused vocab GEMM, batch-pipelined with attention ----
        ab_pend = None
        for b in range(B):
            if b + 2 < B:
                attn_head(b + 2)
            if b + 1 < B:
                attn_tail(b + 1)
            res = mn.tile([P, VS], bf16, tag="res", bufs=2, name=f"res_{b}")
            for g in range(NNT):
                vsl = slice(g * NTILE, (g + 1) * NTILE)
                ps = pp.tile([P, NTILE], f32, tag="big", bufs=4, name=f"ps_{b}_{g}")
                for j in range(KC + 1):
                    if j < KC:
                        rhs = wg_m[:, j, vsl]
                    else:
                        rhs = mb_all[:, b, g // 4, (g % 4) * NTILE : (g % 4 + 1) * NTILE]
                    nc.tensor.matmul(
                        ps[:],
                        lhsT=hT_all[:, b, j, :],
                        rhs=rhs,
                        start=(j == 0),
                        stop=(j == KC and not bg_nonzero),
                    )
                if bg_nonzero:
                    nc.tensor.matmul(
                        ps[:], lhsT=omrow[:, b, :], rhs=bg_m[:, vsl],
                        start=False, stop=True,
                    )
                nc.scalar.copy(res[:, vsl], ps[:])
                if b + 1 < B:
                    if g == 2:
                        ab_pend = gate_zb(b + 1)
                    elif g == 4:
                        gate_abc(b + 1, ab_pend)
                    elif g == 5:
                        build_hT(b + 1)
                if b < B - 1:
                    if g == NNT // 2 - 1:
                        nc.sync.dma_start(
                            out_d[:, b, 0 : VS // 2], res[:, 0 : VS // 2]
                        )
                elif g % 2 == 1:
                    # last batch: quarter DMAs to shrink the drain tail
                    qsl = slice((g - 1) * NTILE, (g + 1) * NTILE)
                    nc.sync.dma_start(out_d[:, b, qsl], res[:, qsl])
            if b < B - 1:
                nc.sync.dma_start(out_d[:, b, VS // 2 : VS], res[:, VS // 2 : VS])

    nc.compile()
    return nc


def _host_prep(inputs):
    htgt = np.asarray(inputs["htgt"], dtype=np.float32)
    hsrc = np.asarray(inputs["hsrc"], dtype=np.float32)
    src = np.asarray(inputs["src"]).astype(np.int64)
    Wq = np.asarray(inputs["Wq"], dtype=np.float32)
    bq = np.asarray(inputs["bq"], dtype=np.float32)
    Wf = np.asarray(inputs["Wf"], dtype=np.float32)
    bf = np.asarray(inputs["bf"], dtype=np.float32)
    Wg = np.asarray(inputs["Wg"], dtype=np.float32)
    bg = np.asarray(inputs["bg"], dtype=np.float32)
    Wc = np.asarray(inputs["Wc"], dtype=np.float32)
    bc = np.asarray(inputs["bc"], dtype=np.float32)

    import ml_dtypes

    bf16 = ml_dtypes.bfloat16

    # Gate weight chain (tiny): w3 = Wq@Wf@Wc, c0 = NT*(bq@Wf@Wc + bf@Wc) + bc
    wfc = (Wf.astype(np.float64) @ Wc.astype(np.float64))[:, 0]      # (D,)
    w3 = (Wq.astype(np.float64) @ wfc).astype(np.float32)            # (D,)
    c0 = float(
        NT * (bq.astype(np.float64) @ wfc)
        + NT * (bf.astype(np.float64) @ Wc.astype(np.float64)[:, 0])
        + bc[0]
    )
    c0v = np.full((P,), c0 / NT, dtype=np.float32)

    # Attention algebra: logits ~ (htgt@M' + bqq) @ hsrc.T  (per-row consts
    # dropped; softmax-invariant). M' = Wq@Wq.T/sqrt(D), bqq = bq@Wq.T/sqrt(D).
    Wq64 = Wq.astype(np.float64)
    Mp = (Wq64 @ Wq64.T * SQ).astype(np.float32)                     # (D, D)
    bqq = (bq.astype(np.float64) @ Wq64.T * SQ).astype(np.float32)   # (D,)

    def pmajor(x):  # (D, ...) -> (P, KC, ...) partition-major
        return np.ascontiguousarray(
            x.reshape((KC, P) + x.shape[1:]).swapaxes(0, 1)
        )

    # h transposes: (N, B, D) -> (D, B, N) -> (P, KC, B, N), cast bf16
    htgtT = pmajor(np.ascontiguousarray(htgt.transpose(2, 1, 0))).astype(bf16)
    hsrcT = pmajor(np.ascontiguousarray(hsrc.transpose(2, 1, 0))).astype(bf16)
    mp = pmajor(Mp).astype(bf16)                                     # (P, KC, D)
    w3p = pmajor(w3).astype(bf16)                                    # (P, KC)
    bqqp = pmajor(bqq).astype(np.float32)                            # (P, KC)
    WgT = pmajor(Wg)                                                 # (P, KC, V)

    bg_nonzero = bool(np.any(bg != 0.0))

    in_maps = []
    for c in range(NCORES):
        v0 = c * VS
        # local_scatter indices: per chunk of 2000 vocab cols, the
        # in-chunk offset of src[s,b] or -1 (ignored); second slot pads
        # num_idxs to an even 2.
        srcidx = np.full((P, B, 2, 2), -1, dtype=np.int16)
        for ch in range(2):
            off = src.astype(np.int64) - v0 - ch * MBC               # (NS, B)
            valid = (off >= 0) & (off < MBC)
            srcidx[:, :, ch, 0] = np.where(valid, off, -1).astype(np.int16)
        in_maps.append(
            {
                "htgtT": htgtT,
                "hsrcT": hsrcT,
                "srcidx": srcidx,
                "mprime": mp,
                "wg": np.ascontiguousarray(WgT[:, :, v0 : v0 + VS]).astype(bf16),
                "w3": w3p,
                "bqq": bqqp,
                "bg": np.ascontiguousarray(bg[v0 : v0 + VS]),
                "c0v": c0v,
            }
        )
    return in_maps, bg_nonzero


TRACE = False
TRACE_KW: dict = {}
LAST_RESULT = None


def kernel(**inputs) -> np.ndarray:
    global LAST_RESULT
    from concourse.bass_utils import run_bass_kernel_spmd

    in_maps, bg_nonzero = _host_prep(inputs)
    key = ("mod", bg_nonzero)
    if key not in _module_cache:
        _module_cache[key] = _build_module(bg_nonzero)
    nc = _module_cache[key]

    r = run_bass_kernel_spmd(
        nc, in_maps, core_ids=list(range(NCORES)), trace=TRACE, **TRACE_KW
    )
    LAST_RESULT = r
    shards = [r.results[c]["out"].astype(np.float32) for c in range(NCORES)]
    return np.concatenate(shards, axis=2)
